# revision 15
# baseline (speedup 1.0000x reference)
"""Causal self-attention with interleaved RoPE on 8 Trainium2 NeuronCores, v2.

Problem: B=4, T=2048, C=1024, H=16, D=64 (fp32 reference).
  qkv = x @ W_in + b_in ; per-head interleaved RoPE on q,k ;
  causal softmax attention ; y @ W_out + b_out.

Sharding: core c <-> (batch b = c//2, head-half = c%2, 8 heads each).
COLLECTIVE-FREE: each core computes attention for its 8 heads over the
full T, normalizes locally (softmax denominators ride the PV matmul as a
ones-column of V), then computes a PARTIAL output projection using only
its heads' rows of W_out for ALL 2048 rows. The host adds the two
per-batch partials during unsharding (f32), which replaces the
AllGather/AllReduce entirely.

Structure (J-waves, J = query block of 512):
  wave J: reverse-project qT,kT (+RoPE via stream_shuffle partition
  swap) and forward-project V for t-rows [512J, 512J+512); attention for
  queries J over key blocks 0..4J+3 with combined 2-i-step [128,1024]
  exps; per-pair local normalize; partial out-projection of rows J.
  Projection/out-projection matmuls are spliced between attention
  i-pairs to keep the PE dense (HAM stays at K=8/8) while the scalar
  engine streams exps.

Layout notes:
 - qT/kT are produced TRANSPOSED directly (stationary = W chunk, moving
   = xT), so there are no PE transposes; RoPE pairs are host-permuted to
   [e0-15, o0-15, e16-31, o16-31] per head so the rotate-half partner
   sits +-16 partitions away inside one 32-partition quadrant
   (stream_shuffle mask [16..31,0..15]).
 - Scores are computed transposed (S^T [tk, tq]); exp without
   max-subtraction (|S|*scale <= ~6); diagonal blocks get full-width S,
   then memset+tri-mask on p (bf16).
 - Softmax denominators: ones-column in V -> PV psum row 64; normalize =
   cross-base row copy + bf16 reciprocal + sel-matmul broadcast + mul.
"""

import numpy as np

B, T, C, H = 4, 2048, 1024, 16
D = C // H            # 64
HPC = H // 2          # 8 heads per core
NP = HPC // 2         # 4 head-pairs per core
N_CORES = 8
ROPE_BASE = 10000.0
NW = 4                # 4 waves of 512 queries

_CACHE = {}

SHUF_MASK = list(range(16, 32)) + list(range(0, 16))


def _build_program(use_bias=False):
    import concourse.bass as bass
    import concourse.bacc as bacc
    import concourse.tile as tile
    import concourse.mybir as mybir

    f32 = mybir.dt.float32
    bf16 = mybir.dt.bfloat16
    Exp = mybir.ActivationFunctionType.Exp
    SCALE = 1.0 / float(np.sqrt(D))

    nc = bacc.Bacc("TRN2", target_bir_lowering=False, debug=False,
                   num_devices=N_CORES)

    xt_d = nc.dram_tensor("xt", [NW, 128, 8, 512], bf16, kind="ExternalInput")
    wqk_d = nc.dram_tensor("wqk", [128, 8, 1024], bf16, kind="ExternalInput")
    wv_d = nc.dram_tensor("wv", [128, 8, 512], bf16, kind="ExternalInput")
    wout_d = nc.dram_tensor("wout", [128, 4, 1024], bf16, kind="ExternalInput")
    cos_d = nc.dram_tensor("cosT", [128, T], bf16, kind="ExternalInput")
    sin_d = nc.dram_tensor("sinT", [128, T], bf16, kind="ExternalInput")
    tri_d = nc.dram_tensor("trimask", [128, 128], bf16, kind="ExternalInput")
    if use_bias:
        wqkb_d = nc.dram_tensor("wqkb", [128, 8], f32, kind="ExternalInput")
        wqkbs_d = nc.dram_tensor("wqkbs", [128, 8], f32, kind="ExternalInput")
        wvb_d = nc.dram_tensor("wvb", [1, 512], bf16, kind="ExternalInput")
    out_d = nc.dram_tensor("out", [T, C], bf16, kind="ExternalOutput")

    with tile.TileContext(nc) as tc:
        with (
            tc.tile_pool(name="g", bufs=1) as g,
            tc.tile_pool(name="xp", bufs=2) as xp,
            tc.tile_pool(name="rp", bufs=3) as rp,
            tc.tile_pool(name="pP", bufs=8) as pP,
            tc.tile_pool(name="yp", bufs=4) as yp,
            tc.tile_pool(name="op", bufs=2) as op,
            tc.tile_pool(name="psS", bufs=2, space="PSUM") as psS,
            tc.tile_pool(name="psV", bufs=2, space="PSUM") as psV,
            tc.tile_pool(name="psW", bufs=2, space="PSUM") as psW,
        ):
            # ---- constants (need-ordered: xt0 + first qk groups first) ----
            wqk_sb = g.tile([128, 8, 1024], bf16)
            xt_t = [None] * NW
            xt_t[0] = None  # placeholder; real alloc below
            cos_sb = g.tile([128, T], bf16)
            sin_sb = g.tile([128, T], bf16)
            wv_sb = g.tile([128, 8, 512], bf16)
            tri_sb = g.tile([128, 128], bf16)
            wout_sb = g.tile([128, 4, 1024], bf16)
            if use_bias:
                wqkb_sb = g.tile([128, 8], f32)
                nc.sync.dma_start(wqkb_sb[:], wqkb_d[:])
                wqkbs_sb = g.tile([128, 8], f32)
                nc.sync.dma_start(wqkbs_sb[:], wqkbs_d[:])
                wvb_sb = g.tile([1, 512], bf16)
                nc.sync.dma_start(wvb_sb[:], wvb_d[:])
                ones1 = g.tile([1, 128], bf16)
                nc.vector.memset(ones1[:], 1.0)

            # sel: rows 0/64 select broadcast sources for the two heads
            sel_sb = g.tile([128, 128], bf16)
            nc.vector.memset(sel_sb[:], 0.0)
            nc.vector.memset(sel_sb[0:1, 0:64], 1.0)
            nc.vector.memset(sel_sb[64:65, 64:128], 1.0)
            # r2: rows 0/64 hold denominators; other rows stay 1.0 so a
            # single reciprocal over [0:65] can't produce inf/nan
            r2_sb = g.tile([128, 512], f32)
            nc.vector.memset(r2_sb[:], 1.0)
            r2b_sb = g.tile([128, 512], bf16)
            nc.vector.memset(r2b_sb[:], 0.0)

            # ---- persistent activations ----
            qt_sb = g.tile([128, NP, T], bf16)
            kt_sb = g.tile([128, NP, T], bf16)
            v_sb = g.tile([128, 16, HPC, 65], bf16)
            nc.vector.memset(v_sb[:, :, :, 64:65], 1.0)

            def load_wave(J):
                xt_t[J] = xp.tile([128, 8, 512], bf16, tag="xt", name=f"xt{J}")
                nc.sync.dma_start(xt_t[J][:, 0:4, :], xt_d[J, :, 0:4, :])
                nc.sync.dma_start(xt_t[J][:, 4:8, :], xt_d[J, :, 4:8, :])
                ts = slice(J * 512, (J + 1) * 512)
                nc.sync.dma_start(cos_sb[:, ts], cos_d[:, ts])
                nc.sync.dma_start(sin_sb[:, ts], sin_d[:, ts])

            _qk_ps = {}

            def qk_group_a(J, gi):
                ps = psW.tile([128, 512], f32, tag="w", name=f"qk{J}_{gi}")
                _qk_ps[(J, gi)] = ps
                for kc in range(4):
                    nc.tensor.matmul(ps[:],
                                     wqk_sb[:, kc, gi * 128:(gi + 1) * 128],
                                     xt_t[J][:, kc, :],
                                     start=(kc == 0), stop=False)

            def qk_group_b(J, gi):
                ps = _qk_ps.pop((J, gi))
                for kc in range(4, 8):
                    nc.tensor.matmul(ps[:],
                                     wqk_sb[:, kc, gi * 128:(gi + 1) * 128],
                                     xt_t[J][:, kc, :],
                                     start=False, stop=(kc == 7))
                ts = slice(J * 512, (J + 1) * 512)
                m1 = rp.tile([128, 512], bf16, tag="m1", name="m1")
                tcos = rp.tile([128, 512], bf16, tag="tc", name="tcos")
                if use_bias:
                    sh = rp.tile([128, 512], f32, tag="sh", name="sh")
                    nc.vector.stream_shuffle(sh[:], ps[:], SHUF_MASK)
                    nc.vector.scalar_tensor_tensor(
                        tcos[:], ps[:], wqkb_sb[:, gi:gi + 1], cos_sb[:, ts],
                        op0=mybir.AluOpType.add, op1=mybir.AluOpType.mult)
                    nc.vector.scalar_tensor_tensor(
                        m1[:], sh[:], wqkbs_sb[:, gi:gi + 1], sin_sb[:, ts],
                        op0=mybir.AluOpType.add, op1=mybir.AluOpType.mult)
                else:
                    # drain psum on the scalar engine; the whole vector
                    # chain then runs at 2x bf16 DVE rate
                    qraw = rp.tile([128, 512], bf16, tag="qr", name="qraw")
                    nc.scalar.copy(qraw[:], ps[:])
                    sh = rp.tile([128, 512], bf16, tag="sh", name="sh")
                    nc.vector.stream_shuffle(sh[:], qraw[:], SHUF_MASK)
                    nc.vector.tensor_mul(tcos[:], qraw[:], cos_sb[:, ts])
                    nc.vector.tensor_mul(m1[:], sh[:], sin_sb[:, ts])
                dst = qt_sb[:, gi, ts] if gi < 4 else kt_sb[:, gi - 4, ts]
                nc.vector.tensor_add(dst, tcos[:], m1[:])

            def qk_group(J, gi):
                qk_group_a(J, gi)
                qk_group_b(J, gi)

            def v_chunk_a(J, ci):
                ps = psW.tile([128, 512], f32, tag="w", name=f"v{J}_{ci}")
                _qk_ps[("v", J, ci)] = ps
                for kc in range(4):
                    nc.tensor.matmul(ps[:],
                                     xt_t[J][:, kc, ci * 128:(ci + 1) * 128],
                                     wv_sb[:, kc, :],
                                     start=(kc == 0), stop=False)

            def v_chunk_b(J, ci):
                ps = _qk_ps.pop(("v", J, ci))
                for kc in range(4, 8):
                    nc.tensor.matmul(ps[:],
                                     xt_t[J][:, kc, ci * 128:(ci + 1) * 128],
                                     wv_sb[:, kc, :],
                                     start=False,
                                     stop=(kc == 7 and not use_bias))
                if use_bias:
                    nc.tensor.matmul(ps[:], ones1[0:1, :], wvb_sb[0:1, :],
                                     start=False, stop=True)
                tb = 4 * J + ci
                nc.vector.tensor_copy(
                    v_sb[:, tb, :, 0:64],
                    ps[:].rearrange("p (h d) -> p h d", d=64))

            def v_chunk(J, ci):
                v_chunk_a(J, ci)
                v_chunk_b(J, ci)

            y_t = [None] * NW

            def outproj_half(J, ci, lohi):
                """Partial out-proj for rows 512J+128ci, output half lohi."""
                ysl = y_t[J]
                pr = psW.tile([128, 512], f32, tag="w", name=f"pr{J}_{ci}_{lohi}")
                cs = slice(lohi * 512, (lohi + 1) * 512)
                for pp in range(NP):
                    nc.tensor.matmul(pr[:],
                                     ysl[:, pp, ci * 128:(ci + 1) * 128],
                                     wout_sb[:, pp, cs],
                                     start=(pp == 0), stop=(pp == NP - 1))
                ob = op.tile([128, 1024], bf16, tag="ob", name="ob")
                nc.vector.tensor_copy(ob[:, cs], pr[:])
                r0 = J * 512 + ci * 128
                nc.sync.dma_start(out_d[r0:r0 + 128, cs], ob[:, cs])
                return ob

            # out-proj halves must share the ob tile; emit as paired fillers
            _ob_pend = {}

            def outproj_chunk_fillers(J):
                fills = []
                for ci in range(4):
                    def f0(J=J, ci=ci):
                        _ob_pend[(J, ci)] = outproj_half(J, ci, 0)
                    def f1(J=J, ci=ci):
                        ysl = y_t[J]
                        ob = _ob_pend.pop((J, ci))
                        pr = psW.tile([128, 512], f32, tag="w",
                                      name=f"pr{J}_{ci}_1")
                        for pp in range(NP):
                            nc.tensor.matmul(pr[:],
                                             ysl[:, pp, ci * 128:(ci + 1) * 128],
                                             wout_sb[:, pp, 512:1024],
                                             start=(pp == 0), stop=(pp == NP - 1))
                        nc.vector.tensor_copy(ob[:, 512:1024], pr[:])
                        r0 = J * 512 + ci * 128
                        nc.sync.dma_start(out_d[r0:r0 + 128, 512:1024],
                                          ob[:, 512:1024])
                    fills.append(f0)
                    fills.append(f1)
                return fills

            def attn_wave(J, fillers):
                n_i = 4 * J + 4
                y_t[J] = yp.tile([128, NP, 512], bf16, tag="y", name=f"y{J}")
                fi = 0
                pend_chain = []
                pend_norm = []
                pend_pv = []
                tot_ipairs = NP * (n_i // 2)
                ip_idx = 1
                pop_at = min(2, n_i // 2 - 1)
                chain_at = max(0, pop_at - 1)
                for pp in range(NP):
                    pv = [psV.tile([128, 512], f32, tag="pv", name=f"pv{hh}")
                          for hh in range(2)]
                    for u in range(n_i // 2):
                        steps = (2 * u, 2 * u + 1)
                        di0 = 2 * u - 4 * J
                        sp = [psS.tile([128, 1024], f32, tag="s", name=f"s{hh}")
                              for hh in range(2)]
                        # hh outer so sp[0] completes 1 matmul earlier
                        for hh in range(2):
                            row = hh * 64
                            for ii, i in enumerate(steps):
                                di = i - 4 * J
                                c0 = di * 128 if di > 0 else 0
                                nc.tensor.matmul(
                                    sp[hh][:, ii * 512 + c0:(ii + 1) * 512],
                                    kt_sb[row:row + 64, pp, i * 128:(i + 1) * 128],
                                    qt_sb[row:row + 64, pp,
                                          J * 512 + c0:(J + 1) * 512],
                                    start=True, stop=True,
                                    tile_position=(row, 0))
                        p_t = [pP.tile([128, 1024], bf16, tag="p",
                                       name=f"p{hh}") for hh in range(2)]
                        for hh in range(2):
                            if di0 == 2:
                                # far-diagonal pair: only unmasked columns
                                nc.scalar.activation(p_t[hh][:, 256:512],
                                                     sp[hh][:, 256:512],
                                                     Exp, scale=SCALE)
                                nc.scalar.activation(p_t[hh][:, 896:1024],
                                                     sp[hh][:, 896:1024],
                                                     Exp, scale=SCALE)
                            else:
                                nc.scalar.activation(p_t[hh][:], sp[hh][:],
                                                     Exp, scale=SCALE)
                        for ii, i in enumerate(steps):
                            di = i - 4 * J
                            if di >= 0:
                                # tri-mask the 128-wide diagonal block
                                for hh in range(2):
                                    dsl = slice(ii * 512 + di * 128,
                                                ii * 512 + (di + 1) * 128)
                                    nc.gpsimd.tensor_mul(p_t[hh][:, dsl],
                                                         p_t[hh][:, dsl],
                                                         tri_sb[:])
                        # PV of the PREVIOUS i-pair: its exp+mask are
                        # long done, so the PE never stalls on them
                        if pend_pv:
                            pend_pv.pop(0)()
                        if u == chain_at and pend_chain:
                            pend_chain.pop(0)()
                        if u == pop_at and pend_norm:
                            pend_norm.pop(0)()
                        while (fi < len(fillers)
                               and fi * tot_ipairs <= ip_idx * len(fillers)):
                            fillers[fi]()
                            fi += 1
                        ip_idx += 1

                        def _pv(pp=pp, u=u, steps=steps, pv=pv, p_t=p_t,
                                n_i=n_i, J=J):
                            for ii, i in enumerate(steps):
                                di = i - 4 * J
                                c0 = di * 128 if di > 0 else 0
                                for hh in range(2):
                                    h = 2 * pp + hh
                                    nc.tensor.matmul(
                                        pv[hh][0:65, c0:512],
                                        v_sb[:, i, h, 0:65],
                                        p_t[hh][:, ii * 512 + c0:(ii + 1) * 512],
                                        start=(u == 0 and ii == 0),
                                        stop=(u == n_i // 2 - 1 and ii == 1))
                        pend_pv.append(_pv)
                    # ---- normalize pair pp (deferred emission: the
                    # sel-matmul lands after the next pair's first S block
                    # so the PE never stalls on the reciprocal chain) ----
                    def _norm_chain(J=J, pp=pp, pv=pv):
                        nc.scalar.copy(r2_sb[0:1, :], pv[0][64:65, :])
                        nc.scalar.copy(r2_sb[64:65, :], pv[1][64:65, :])
                        nc.vector.reciprocal_approx_fast(r2_sb[0:65, :],
                                                         r2_sb[0:65, :])
                        nc.vector.tensor_copy(r2b_sb[0:65, :], r2_sb[0:65, :])
                    def _norm_bcast(J=J, pp=pp, pv=pv):
                        rf = psW.tile([128, 512], f32, tag="w", name="rf")
                        nc.tensor.matmul(rf[:], sel_sb[0:65, :],
                                         r2b_sb[0:65, :],
                                         start=True, stop=True)
                        rfs = rp.tile([128, 512], bf16, tag="rfs", bufs=2,
                                      name="rfs")
                        nc.scalar.copy(rfs[:], rf[:])
                        nc.vector.tensor_mul(y_t[J][0:64, pp, :],
                                             pv[0][0:64, :], rfs[0:64, :])
                        nc.vector.tensor_mul(y_t[J][64:128, pp, :],
                                             pv[1][0:64, :], rfs[64:128, :])
                    pend_chain.append(_norm_chain)
                    pend_norm.append(_norm_bcast)
                # drain deferred PV, then leftover fillers, then norms
                while pend_pv:
                    pend_pv.pop(0)()
                while pend_chain:
                    pend_chain.pop(0)()
                while fi < len(fillers):
                    fillers[fi]()
                    fi += 1
                while pend_norm:
                    pend_norm.pop(0)()

            # ================= emission =================
            # wave 0: project only pair 0 + V upfront; pairs 1-3 become the
            # leading fillers of attn(0) (each pair's groups land one pair
            # ahead of use). outproj(J) runs as fillers in wave J+2 (y pool
            # bufs=3 allows the two-wave lag) so wave 3 stays fed.
            # head: need-ordered chunked DMAs so the first matmul can
            # start as soon as wqk g0/g4 chunks + xt0 kc chunks land.
            xt_t[0] = xp.tile([128, 8, 512], bf16, tag="xt", name="xt0")
            nc.sync.dma_start(wqk_sb[:, 0:4, 0:128], wqk_d[:, 0:4, 0:128])
            nc.sync.dma_start(xt_t[0][:, 0:4, :], xt_d[0, :, 0:4, :])
            nc.sync.dma_start(wqk_sb[:, 0:4, 512:640], wqk_d[:, 0:4, 512:640])
            nc.sync.dma_start(wqk_sb[:, 4:8, 0:128], wqk_d[:, 4:8, 0:128])
            nc.sync.dma_start(wqk_sb[:, 4:8, 512:640], wqk_d[:, 4:8, 512:640])
            nc.sync.dma_start(xt_t[0][:, 4:8, :], xt_d[0, :, 4:8, :])
            nc.sync.dma_start(cos_sb[:, 0:512], cos_d[:, 0:512])
            nc.sync.dma_start(sin_sb[:, 0:512], sin_d[:, 0:512])
            nc.sync.dma_start(wv_sb[:, 0:4, :], wv_d[:, 0:4, :])
            nc.sync.dma_start(wv_sb[:, 4:8, :], wv_d[:, 4:8, :])
            nc.sync.dma_start(tri_sb[:], tri_d[:])
            nc.sync.dma_start(wqk_sb[:, :, 128:512], wqk_d[:, :, 128:512])
            nc.sync.dma_start(wqk_sb[:, :, 640:1024], wqk_d[:, :, 640:1024])
            qk_group(0, 0)
            qk_group(0, 4)
            v_chunk(0, 0)
            v_chunk(0, 1)
            nc.sync.dma_start(wout_sb[:], wout_d[:])

            def proj_fillers(J):
                fills = []
                for gi in range(8):
                    fills.append(lambda J=J, gi=gi: qk_group_a(J, gi))
                    fills.append(lambda J=J, gi=gi: qk_group_b(J, gi))
                for ci in range(4):
                    fills.append(lambda J=J, ci=ci: v_chunk_a(J, ci))
                    fills.append(lambda J=J, ci=ci: v_chunk_b(J, ci))
                return fills

            for J in range(NW):
                fillers = []
                if J == 0:
                    for ci in (2, 3):
                        fillers.append(lambda ci=ci: v_chunk_a(0, ci))
                        fillers.append(lambda ci=ci: v_chunk_b(0, ci))
                    for gi in (1, 5, 2, 6, 3, 7):
                        fillers.append(lambda gi=gi: qk_group_a(0, gi))
                        fillers.append(lambda gi=gi: qk_group_b(0, gi))
                if J == NW - 1:
                    for Jo in range(NW - 1):
                        fillers += outproj_chunk_fillers(Jo)
                if J < NW - 1:
                    load_wave(J + 1)
                    fillers += proj_fillers(J + 1)
                attn_wave(J, fillers)
            for f in outproj_chunk_fillers(NW - 1):
                f()

    nc.compile()
    return nc


def _host_prep(x, W_in, b_in, W_out):
    """Build per-core input maps."""
    import ml_dtypes

    bf = ml_dtypes.bfloat16

    # RoPE pair permutation: [e0..e15, o0..o15, e16..e31, o16..o31]
    perm = np.empty(D, np.int64)
    sign = np.empty(D, np.float64)
    fidx = np.empty(D, np.int64)
    for d in range(D):
        qd, w = d // 32, d % 32
        f = qd * 16 + (w % 16)
        perm[d] = 2 * f + (0 if w < 16 else 1)
        sign[d] = -1.0 if w < 16 else 1.0
        fidx[d] = f
    inv_freq = 1.0 / (ROPE_BASE ** (np.arange(0, D, 2, dtype=np.float64) / D))
    tpos = np.arange(T, dtype=np.float64)
    ang = tpos[None, :] * inv_freq[fidx][:, None]          # [64, T]
    cosT = np.tile(np.cos(ang), (2, 1)).astype(bf)  # [128, T]
    sinT = np.tile(sign[:, None] * np.sin(ang), (2, 1)).astype(bf)

    tri = (np.arange(128)[None, :] >= np.arange(128)[:, None]).astype(bf)

    in_maps = []
    for c in range(N_CORES):
        b, half = c // 2, c % 2
        heads = np.arange(half * HPC, (half + 1) * HPC)

        xt = np.ascontiguousarray(
            x[b].astype(bf).reshape(4, 512, 8, 128).transpose(0, 3, 2, 1))

        qk_cols = []
        for gi in range(4):
            hA, hB = heads[2 * gi], heads[2 * gi + 1]
            qk_cols.append(hA * D + perm)
            qk_cols.append(hB * D + perm)
        qcols = np.concatenate(qk_cols)
        kcols = C + qcols
        allqk = np.concatenate([qcols, kcols])
        wqk = np.ascontiguousarray(
            W_in[:, allqk].astype(bf).reshape(8, 128, 1024).transpose(1, 0, 2))
        vcols = np.concatenate([2 * C + h * D + np.arange(D) for h in heads])
        wv = np.ascontiguousarray(
            W_in[:, vcols].astype(bf).reshape(8, 128, 512).transpose(1, 0, 2))
        wout = np.ascontiguousarray(
            W_out[half * 512:(half + 1) * 512, :]
            .astype(bf).reshape(4, 128, 1024).transpose(1, 0, 2))

        m = {
            "xt": xt, "wqk": wqk, "wv": wv, "wout": wout,
            "cosT": cosT, "sinT": sinT, "trimask": tri,
        }
        if np.any(b_in != 0):
            qb = b_in[qcols]
            kb = b_in[kcols]
            wqkb = np.empty((128, 8), np.float32)
            wqkbs = np.empty((128, 8), np.float32)
            swap = np.concatenate([np.arange(16, 32), np.arange(0, 16),
                                   np.arange(48, 64), np.arange(32, 48)])
            for gi in range(8):
                bias = (qb if gi < 4 else kb)[(gi % 4) * 128:(gi % 4 + 1) * 128]
                wqkb[:, gi] = bias
                wqkbs[:, gi] = bias[swap]
            m["wqkb"] = wqkb
            m["wqkbs"] = wqkbs
            m["wvb"] = b_in[None, vcols].astype(bf)
        in_maps.append(m)
    return in_maps


LAST_RESULT = None


def kernel(x, W_in, b_in, W_out, b_out, _trace=False):
    global LAST_RESULT
    from concourse.bass_utils import run_bass_kernel_spmd

    x = np.asarray(x, dtype=np.float32)
    W_in = np.asarray(W_in, dtype=np.float32)
    b_in = np.asarray(b_in, dtype=np.float32)
    W_out = np.asarray(W_out, dtype=np.float32)
    b_out = np.asarray(b_out, dtype=np.float32)

    use_bias = bool(np.any(b_in != 0))
    key = ("nc", use_bias)
    if key not in _CACHE:
        _CACHE[key] = _build_program(use_bias=use_bias)
    nc = _CACHE[key]

    in_maps = _host_prep(x, W_in, b_in, W_out)
    res = run_bass_kernel_spmd(nc, in_maps, core_ids=list(range(N_CORES)),
                               trace=_trace)
    LAST_RESULT = res

    out = np.empty((B, T, C), np.float32)
    for b in range(B):
        out[b] = (res.results[2 * b]["out"].astype(np.float32)
                  + res.results[2 * b + 1]["out"].astype(np.float32))
    if np.any(b_out != 0):
        out = out + b_out[None, None, :]
    return out



# revision 16
# speedup vs baseline: 1.0217x; 1.0217x over previous
"""Causal self-attention with interleaved RoPE on 8 Trainium2 NeuronCores, v2.

Problem: B=4, T=2048, C=1024, H=16, D=64 (fp32 reference).
  qkv = x @ W_in + b_in ; per-head interleaved RoPE on q,k ;
  causal softmax attention ; y @ W_out + b_out.

Sharding: core c <-> (batch b = c//2, head-half = c%2, 8 heads each).
COLLECTIVE-FREE: each core computes attention for its 8 heads over the
full T, normalizes locally (softmax denominators ride the PV matmul as a
ones-column of V), then computes a PARTIAL output projection using only
its heads' rows of W_out for ALL 2048 rows. The host adds the two
per-batch partials during unsharding (f32), which replaces the
AllGather/AllReduce entirely.

Structure (J-waves, J = query block of 512):
  wave J: reverse-project qT,kT (+RoPE via stream_shuffle partition
  swap) and forward-project V for t-rows [512J, 512J+512); attention for
  queries J over key blocks 0..4J+3 with combined 2-i-step [128,1024]
  exps; per-pair local normalize; partial out-projection of rows J.
  Projection/out-projection matmuls are spliced between attention
  i-pairs to keep the PE dense (HAM stays at K=8/8) while the scalar
  engine streams exps.

Layout notes:
 - qT/kT are produced TRANSPOSED directly (stationary = W chunk, moving
   = xT), so there are no PE transposes; RoPE pairs are host-permuted to
   [e0-15, o0-15, e16-31, o16-31] per head so the rotate-half partner
   sits +-16 partitions away inside one 32-partition quadrant
   (stream_shuffle mask [16..31,0..15]).
 - Scores are computed transposed (S^T [tk, tq]); exp without
   max-subtraction (|S|*scale <= ~6); diagonal blocks get full-width S,
   then memset+tri-mask on p (bf16).
 - Softmax denominators: ones-column in V -> PV psum row 64; normalize =
   cross-base row copy + bf16 reciprocal + sel-matmul broadcast + mul.
"""

import numpy as np

B, T, C, H = 4, 2048, 1024, 16
D = C // H            # 64
HPC = H // 2          # 8 heads per core
NP = HPC // 2         # 4 head-pairs per core
N_CORES = 8
ROPE_BASE = 10000.0
NW = 4                # 4 waves of 512 queries

_CACHE = {}

SHUF_MASK = list(range(16, 32)) + list(range(0, 16))


def _build_program(use_bias=False):
    import concourse.bass as bass
    import concourse.bacc as bacc
    import concourse.tile as tile
    import concourse.mybir as mybir

    f32 = mybir.dt.float32
    bf16 = mybir.dt.bfloat16
    Exp = mybir.ActivationFunctionType.Exp
    SCALE = 1.0 / float(np.sqrt(D))

    nc = bacc.Bacc("TRN2", target_bir_lowering=False, debug=False,
                   num_devices=N_CORES)

    xt_d = nc.dram_tensor("xt", [NW, 128, 8, 512], bf16, kind="ExternalInput")
    wqk_d = nc.dram_tensor("wqk", [128, 8, 1024], bf16, kind="ExternalInput")
    wv_d = nc.dram_tensor("wv", [128, 8, 512], bf16, kind="ExternalInput")
    wout_d = nc.dram_tensor("wout", [128, 4, 1024], bf16, kind="ExternalInput")
    cos_d = nc.dram_tensor("cosT", [128, T], bf16, kind="ExternalInput")
    sin_d = nc.dram_tensor("sinT", [128, T], bf16, kind="ExternalInput")
    tri_d = nc.dram_tensor("trimask", [128, 128], bf16, kind="ExternalInput")
    if use_bias:
        wqkb_d = nc.dram_tensor("wqkb", [128, 8], f32, kind="ExternalInput")
        wqkbs_d = nc.dram_tensor("wqkbs", [128, 8], f32, kind="ExternalInput")
        wvb_d = nc.dram_tensor("wvb", [1, 512], bf16, kind="ExternalInput")
    out_d = nc.dram_tensor("out", [T, C], bf16, kind="ExternalOutput")

    with tile.TileContext(nc) as tc:
        with (
            tc.tile_pool(name="g", bufs=1) as g,
            tc.tile_pool(name="xp", bufs=2) as xp,
            tc.tile_pool(name="rp", bufs=3) as rp,
            tc.tile_pool(name="pP", bufs=8) as pP,
            tc.tile_pool(name="yp", bufs=4) as yp,
            tc.tile_pool(name="op", bufs=2) as op,
            tc.tile_pool(name="psS", bufs=2, space="PSUM") as psS,
            tc.tile_pool(name="psV", bufs=2, space="PSUM") as psV,
            tc.tile_pool(name="psW", bufs=2, space="PSUM") as psW,
        ):
            # ---- constants (need-ordered: xt0 + first qk groups first) ----
            wqk_sb = g.tile([128, 8, 1024], bf16)
            xt_t = [None] * NW
            xt_t[0] = None  # placeholder; real alloc below
            cos_sb = g.tile([128, T], bf16)
            sin_sb = g.tile([128, T], bf16)
            wv_sb = g.tile([128, 8, 512], bf16)
            tri_sb = g.tile([128, 128], bf16)
            wout_sb = g.tile([128, 4, 1024], bf16)
            if use_bias:
                wqkb_sb = g.tile([128, 8], f32)
                nc.sync.dma_start(wqkb_sb[:], wqkb_d[:])
                wqkbs_sb = g.tile([128, 8], f32)
                nc.sync.dma_start(wqkbs_sb[:], wqkbs_d[:])
                wvb_sb = g.tile([1, 512], bf16)
                nc.sync.dma_start(wvb_sb[:], wvb_d[:])
                ones1 = g.tile([1, 128], bf16)
                nc.vector.memset(ones1[:], 1.0)

            # sel: rows 0/64 select broadcast sources for the two heads
            sel_sb = g.tile([128, 128], bf16)
            nc.vector.memset(sel_sb[:], 0.0)
            nc.vector.memset(sel_sb[0:1, 0:64], 1.0)
            nc.vector.memset(sel_sb[64:65, 64:128], 1.0)
            # r2: rows 0/64 hold denominators; other rows stay 1.0 so a
            # single reciprocal over [0:65] can't produce inf/nan
            r2_sb = g.tile([128, 512], f32)
            nc.vector.memset(r2_sb[:], 1.0)
            r2b_sb = g.tile([128, 512], bf16)
            nc.vector.memset(r2b_sb[:], 0.0)

            # ---- persistent activations ----
            qt_sb = g.tile([128, NP, T], bf16)
            kt_sb = g.tile([128, NP, T], bf16)
            v_sb = g.tile([128, 16, HPC, 65], bf16)
            nc.vector.memset(v_sb[:, :, :, 64:65], 1.0)

            def load_wave(J):
                xt_t[J] = xp.tile([128, 8, 512], bf16, tag="xt", name=f"xt{J}")
                nc.sync.dma_start(xt_t[J][:, 0:4, :], xt_d[J, :, 0:4, :])
                nc.sync.dma_start(xt_t[J][:, 4:8, :], xt_d[J, :, 4:8, :])
                ts = slice(J * 512, (J + 1) * 512)
                nc.sync.dma_start(cos_sb[:, ts], cos_d[:, ts])
                nc.sync.dma_start(sin_sb[:, ts], sin_d[:, ts])

            _qk_ps = {}

            def qk_group_a(J, gi):
                ps = psW.tile([128, 512], f32, tag="w", name=f"qk{J}_{gi}")
                _qk_ps[(J, gi)] = ps
                for kc in range(4):
                    nc.tensor.matmul(ps[:],
                                     wqk_sb[:, kc, gi * 128:(gi + 1) * 128],
                                     xt_t[J][:, kc, :],
                                     start=(kc == 0), stop=False)

            def qk_group_b(J, gi):
                ps = _qk_ps.pop((J, gi))
                for kc in range(4, 8):
                    nc.tensor.matmul(ps[:],
                                     wqk_sb[:, kc, gi * 128:(gi + 1) * 128],
                                     xt_t[J][:, kc, :],
                                     start=False, stop=(kc == 7))
                ts = slice(J * 512, (J + 1) * 512)
                m1 = rp.tile([128, 512], bf16, tag="m1", name="m1")
                tcos = rp.tile([128, 512], bf16, tag="tc", name="tcos")
                if use_bias:
                    sh = rp.tile([128, 512], f32, tag="sh", name="sh")
                    nc.vector.stream_shuffle(sh[:], ps[:], SHUF_MASK)
                    nc.vector.scalar_tensor_tensor(
                        tcos[:], ps[:], wqkb_sb[:, gi:gi + 1], cos_sb[:, ts],
                        op0=mybir.AluOpType.add, op1=mybir.AluOpType.mult)
                    nc.vector.scalar_tensor_tensor(
                        m1[:], sh[:], wqkbs_sb[:, gi:gi + 1], sin_sb[:, ts],
                        op0=mybir.AluOpType.add, op1=mybir.AluOpType.mult)
                else:
                    sh = rp.tile([128, 512], f32, tag="sh", name="sh")
                    nc.vector.stream_shuffle(sh[:], ps[:], SHUF_MASK)
                    nc.vector.tensor_mul(tcos[:], ps[:], cos_sb[:, ts])
                    nc.vector.tensor_mul(m1[:], sh[:], sin_sb[:, ts])
                dst = qt_sb[:, gi, ts] if gi < 4 else kt_sb[:, gi - 4, ts]
                nc.vector.tensor_add(dst, tcos[:], m1[:])

            def qk_group(J, gi):
                qk_group_a(J, gi)
                qk_group_b(J, gi)

            def v_chunk_a(J, ci):
                ps = psW.tile([128, 512], f32, tag="w", name=f"v{J}_{ci}")
                _qk_ps[("v", J, ci)] = ps
                for kc in range(4):
                    nc.tensor.matmul(ps[:],
                                     xt_t[J][:, kc, ci * 128:(ci + 1) * 128],
                                     wv_sb[:, kc, :],
                                     start=(kc == 0), stop=False)

            def v_chunk_b(J, ci):
                ps = _qk_ps.pop(("v", J, ci))
                for kc in range(4, 8):
                    nc.tensor.matmul(ps[:],
                                     xt_t[J][:, kc, ci * 128:(ci + 1) * 128],
                                     wv_sb[:, kc, :],
                                     start=False,
                                     stop=(kc == 7 and not use_bias))
                if use_bias:
                    nc.tensor.matmul(ps[:], ones1[0:1, :], wvb_sb[0:1, :],
                                     start=False, stop=True)
                tb = 4 * J + ci
                nc.vector.tensor_copy(
                    v_sb[:, tb, :, 0:64],
                    ps[:].rearrange("p (h d) -> p h d", d=64))

            def v_chunk(J, ci):
                v_chunk_a(J, ci)
                v_chunk_b(J, ci)

            y_t = [None] * NW

            def outproj_half(J, ci, lohi):
                """Partial out-proj for rows 512J+128ci, output half lohi."""
                ysl = y_t[J]
                pr = psW.tile([128, 512], f32, tag="w", name=f"pr{J}_{ci}_{lohi}")
                cs = slice(lohi * 512, (lohi + 1) * 512)
                for pp in range(NP):
                    nc.tensor.matmul(pr[:],
                                     ysl[:, pp, ci * 128:(ci + 1) * 128],
                                     wout_sb[:, pp, cs],
                                     start=(pp == 0), stop=(pp == NP - 1))
                ob = op.tile([128, 1024], bf16, tag="ob", name="ob")
                nc.vector.tensor_copy(ob[:, cs], pr[:])
                r0 = J * 512 + ci * 128
                nc.sync.dma_start(out_d[r0:r0 + 128, cs], ob[:, cs])
                return ob

            # out-proj halves must share the ob tile; emit as paired fillers
            _ob_pend = {}

            def outproj_chunk_fillers(J):
                fills = []
                for ci in range(4):
                    def f0(J=J, ci=ci):
                        _ob_pend[(J, ci)] = outproj_half(J, ci, 0)
                    def f1(J=J, ci=ci):
                        ysl = y_t[J]
                        ob = _ob_pend.pop((J, ci))
                        pr = psW.tile([128, 512], f32, tag="w",
                                      name=f"pr{J}_{ci}_1")
                        for pp in range(NP):
                            nc.tensor.matmul(pr[:],
                                             ysl[:, pp, ci * 128:(ci + 1) * 128],
                                             wout_sb[:, pp, 512:1024],
                                             start=(pp == 0), stop=(pp == NP - 1))
                        nc.vector.tensor_copy(ob[:, 512:1024], pr[:])
                        r0 = J * 512 + ci * 128
                        nc.sync.dma_start(out_d[r0:r0 + 128, 512:1024],
                                          ob[:, 512:1024])
                    fills.append(f0)
                    fills.append(f1)
                return fills

            def attn_wave(J, fillers):
                n_i = 4 * J + 4
                y_t[J] = yp.tile([128, NP, 512], bf16, tag="y", name=f"y{J}")
                fi = 0
                pend_chain = []
                pend_norm = []
                pend_pv = []
                tot_ipairs = NP * (n_i // 2)
                ip_idx = 1
                pop_at = min(2, n_i // 2 - 1)
                chain_at = max(0, pop_at - 1)
                for pp in range(NP):
                    pv = [psV.tile([128, 512], f32, tag="pv", name=f"pv{hh}")
                          for hh in range(2)]
                    for u in range(n_i // 2):
                        steps = (2 * u, 2 * u + 1)
                        di0 = 2 * u - 4 * J
                        # PV of the PREVIOUS i-pair first: its exp+mask
                        # are done, so the PE has guaranteed-ready work while
                        # this i-pair's scores may still wait on psS reuse
                        if pend_pv:
                            pend_pv.pop(0)()
                        sp = [psS.tile([128, 1024], f32, tag="s", name=f"s{hh}")
                              for hh in range(2)]
                        # hh outer so sp[0] completes 1 matmul earlier
                        for hh in range(2):
                            row = hh * 64
                            for ii, i in enumerate(steps):
                                di = i - 4 * J
                                c0 = di * 128 if di > 0 else 0
                                nc.tensor.matmul(
                                    sp[hh][:, ii * 512 + c0:(ii + 1) * 512],
                                    kt_sb[row:row + 64, pp, i * 128:(i + 1) * 128],
                                    qt_sb[row:row + 64, pp,
                                          J * 512 + c0:(J + 1) * 512],
                                    start=True, stop=True,
                                    tile_position=(row, 0))
                        p_t = [pP.tile([128, 1024], bf16, tag="p",
                                       name=f"p{hh}") for hh in range(2)]
                        for hh in range(2):
                            if di0 == 2:
                                # far-diagonal pair: only unmasked columns
                                nc.scalar.activation(p_t[hh][:, 256:512],
                                                     sp[hh][:, 256:512],
                                                     Exp, scale=SCALE)
                                nc.scalar.activation(p_t[hh][:, 896:1024],
                                                     sp[hh][:, 896:1024],
                                                     Exp, scale=SCALE)
                            else:
                                nc.scalar.activation(p_t[hh][:], sp[hh][:],
                                                     Exp, scale=SCALE)
                        for ii, i in enumerate(steps):
                            di = i - 4 * J
                            if di >= 0:
                                # tri-mask the 128-wide diagonal block
                                for hh in range(2):
                                    dsl = slice(ii * 512 + di * 128,
                                                ii * 512 + (di + 1) * 128)
                                    nc.gpsimd.tensor_mul(p_t[hh][:, dsl],
                                                         p_t[hh][:, dsl],
                                                         tri_sb[:])
                        if u == chain_at and pend_chain:
                            pend_chain.pop(0)()
                        if u == pop_at and pend_norm:
                            pend_norm.pop(0)()
                        while (fi < len(fillers)
                               and fi * tot_ipairs <= ip_idx * len(fillers)):
                            fillers[fi]()
                            fi += 1
                        ip_idx += 1

                        def _pv(pp=pp, u=u, steps=steps, pv=pv, p_t=p_t,
                                n_i=n_i, J=J):
                            for ii, i in enumerate(steps):
                                di = i - 4 * J
                                c0 = di * 128 if di > 0 else 0
                                for hh in range(2):
                                    h = 2 * pp + hh
                                    nc.tensor.matmul(
                                        pv[hh][0:65, c0:512],
                                        v_sb[:, i, h, 0:65],
                                        p_t[hh][:, ii * 512 + c0:(ii + 1) * 512],
                                        start=(u == 0 and ii == 0),
                                        stop=(u == n_i // 2 - 1 and ii == 1))
                        pend_pv.append(_pv)
                    # ---- normalize pair pp (deferred emission: the
                    # sel-matmul lands after the next pair's first S block
                    # so the PE never stalls on the reciprocal chain) ----
                    def _norm_chain(J=J, pp=pp, pv=pv):
                        nc.vector.tensor_copy(r2_sb[0:1, :], pv[0][64:65, :])
                        nc.vector.tensor_copy(r2_sb[64:65, :], pv[1][64:65, :])
                        nc.vector.reciprocal_approx_fast(r2_sb[0:65, :],
                                                         r2_sb[0:65, :])
                        nc.vector.tensor_copy(r2b_sb[0:65, :], r2_sb[0:65, :])
                    def _norm_bcast(J=J, pp=pp, pv=pv):
                        rf = psW.tile([128, 512], f32, tag="w", name="rf")
                        nc.tensor.matmul(rf[:], sel_sb[0:65, :],
                                         r2b_sb[0:65, :],
                                         start=True, stop=True)
                        rfs = rp.tile([128, 512], bf16, tag="rfs", bufs=2,
                                      name="rfs")
                        nc.scalar.copy(rfs[:], rf[:])
                        nc.vector.tensor_mul(y_t[J][0:64, pp, :],
                                             pv[0][0:64, :], rfs[0:64, :])
                        nc.vector.tensor_mul(y_t[J][64:128, pp, :],
                                             pv[1][0:64, :], rfs[64:128, :])
                    pend_chain.append(_norm_chain)
                    pend_norm.append(_norm_bcast)
                # drain deferred PV, then leftover fillers, then norms
                while pend_pv:
                    pend_pv.pop(0)()
                while pend_chain:
                    pend_chain.pop(0)()
                while fi < len(fillers):
                    fillers[fi]()
                    fi += 1
                while pend_norm:
                    pend_norm.pop(0)()

            # ================= emission =================
            # wave 0: project only pair 0 + V upfront; pairs 1-3 become the
            # leading fillers of attn(0) (each pair's groups land one pair
            # ahead of use). outproj(J) runs as fillers in wave J+2 (y pool
            # bufs=3 allows the two-wave lag) so wave 3 stays fed.
            # head: need-ordered chunked DMAs so the first matmul can
            # start as soon as wqk g0/g4 chunks + xt0 kc chunks land.
            xt_t[0] = xp.tile([128, 8, 512], bf16, tag="xt", name="xt0")
            nc.sync.dma_start(wqk_sb[:, 0:4, 0:128], wqk_d[:, 0:4, 0:128])
            nc.sync.dma_start(xt_t[0][:, 0:4, :], xt_d[0, :, 0:4, :])
            nc.sync.dma_start(wqk_sb[:, 0:4, 512:640], wqk_d[:, 0:4, 512:640])
            nc.sync.dma_start(wqk_sb[:, 4:8, 0:128], wqk_d[:, 4:8, 0:128])
            nc.sync.dma_start(wqk_sb[:, 4:8, 512:640], wqk_d[:, 4:8, 512:640])
            nc.sync.dma_start(xt_t[0][:, 4:8, :], xt_d[0, :, 4:8, :])
            nc.sync.dma_start(cos_sb[:, 0:512], cos_d[:, 0:512])
            nc.sync.dma_start(sin_sb[:, 0:512], sin_d[:, 0:512])
            nc.sync.dma_start(wv_sb[:, 0:4, :], wv_d[:, 0:4, :])
            nc.sync.dma_start(wv_sb[:, 4:8, :], wv_d[:, 4:8, :])
            nc.sync.dma_start(tri_sb[:], tri_d[:])
            nc.sync.dma_start(wqk_sb[:, :, 128:512], wqk_d[:, :, 128:512])
            nc.sync.dma_start(wqk_sb[:, :, 640:1024], wqk_d[:, :, 640:1024])
            qk_group(0, 0)
            qk_group(0, 4)
            v_chunk(0, 0)
            v_chunk(0, 1)
            nc.sync.dma_start(wout_sb[:], wout_d[:])

            def proj_fillers(J):
                fills = []
                for gi in range(8):
                    fills.append(lambda J=J, gi=gi: qk_group_a(J, gi))
                    fills.append(lambda J=J, gi=gi: qk_group_b(J, gi))
                for ci in range(4):
                    fills.append(lambda J=J, ci=ci: v_chunk_a(J, ci))
                    fills.append(lambda J=J, ci=ci: v_chunk_b(J, ci))
                return fills

            for J in range(NW):
                fillers = []
                if J == 0:
                    for ci in (2, 3):
                        fillers.append(lambda ci=ci: v_chunk_a(0, ci))
                        fillers.append(lambda ci=ci: v_chunk_b(0, ci))
                    for gi in (1, 5, 2, 6, 3, 7):
                        fillers.append(lambda gi=gi: qk_group_a(0, gi))
                        fillers.append(lambda gi=gi: qk_group_b(0, gi))
                if J == NW - 1:
                    for Jo in range(NW - 1):
                        fillers += outproj_chunk_fillers(Jo)
                if J < NW - 1:
                    load_wave(J + 1)
                    fillers += proj_fillers(J + 1)
                attn_wave(J, fillers)
            for f in outproj_chunk_fillers(NW - 1):
                f()

    nc.compile()
    return nc


def _host_prep(x, W_in, b_in, W_out):
    """Build per-core input maps."""
    import ml_dtypes

    bf = ml_dtypes.bfloat16

    # RoPE pair permutation: [e0..e15, o0..o15, e16..e31, o16..o31]
    perm = np.empty(D, np.int64)
    sign = np.empty(D, np.float64)
    fidx = np.empty(D, np.int64)
    for d in range(D):
        qd, w = d // 32, d % 32
        f = qd * 16 + (w % 16)
        perm[d] = 2 * f + (0 if w < 16 else 1)
        sign[d] = -1.0 if w < 16 else 1.0
        fidx[d] = f
    inv_freq = 1.0 / (ROPE_BASE ** (np.arange(0, D, 2, dtype=np.float64) / D))
    tpos = np.arange(T, dtype=np.float64)
    ang = tpos[None, :] * inv_freq[fidx][:, None]          # [64, T]
    cosT = np.tile(np.cos(ang), (2, 1)).astype(bf)  # [128, T]
    sinT = np.tile(sign[:, None] * np.sin(ang), (2, 1)).astype(bf)

    tri = (np.arange(128)[None, :] >= np.arange(128)[:, None]).astype(bf)

    in_maps = []
    for c in range(N_CORES):
        b, half = c // 2, c % 2
        heads = np.arange(half * HPC, (half + 1) * HPC)

        xt = np.ascontiguousarray(
            x[b].astype(bf).reshape(4, 512, 8, 128).transpose(0, 3, 2, 1))

        qk_cols = []
        for gi in range(4):
            hA, hB = heads[2 * gi], heads[2 * gi + 1]
            qk_cols.append(hA * D + perm)
            qk_cols.append(hB * D + perm)
        qcols = np.concatenate(qk_cols)
        kcols = C + qcols
        allqk = np.concatenate([qcols, kcols])
        wqk = np.ascontiguousarray(
            W_in[:, allqk].astype(bf).reshape(8, 128, 1024).transpose(1, 0, 2))
        vcols = np.concatenate([2 * C + h * D + np.arange(D) for h in heads])
        wv = np.ascontiguousarray(
            W_in[:, vcols].astype(bf).reshape(8, 128, 512).transpose(1, 0, 2))
        wout = np.ascontiguousarray(
            W_out[half * 512:(half + 1) * 512, :]
            .astype(bf).reshape(4, 128, 1024).transpose(1, 0, 2))

        m = {
            "xt": xt, "wqk": wqk, "wv": wv, "wout": wout,
            "cosT": cosT, "sinT": sinT, "trimask": tri,
        }
        if np.any(b_in != 0):
            qb = b_in[qcols]
            kb = b_in[kcols]
            wqkb = np.empty((128, 8), np.float32)
            wqkbs = np.empty((128, 8), np.float32)
            swap = np.concatenate([np.arange(16, 32), np.arange(0, 16),
                                   np.arange(48, 64), np.arange(32, 48)])
            for gi in range(8):
                bias = (qb if gi < 4 else kb)[(gi % 4) * 128:(gi % 4 + 1) * 128]
                wqkb[:, gi] = bias
                wqkbs[:, gi] = bias[swap]
            m["wqkb"] = wqkb
            m["wqkbs"] = wqkbs
            m["wvb"] = b_in[None, vcols].astype(bf)
        in_maps.append(m)
    return in_maps


LAST_RESULT = None


def kernel(x, W_in, b_in, W_out, b_out, _trace=False):
    global LAST_RESULT
    from concourse.bass_utils import run_bass_kernel_spmd

    x = np.asarray(x, dtype=np.float32)
    W_in = np.asarray(W_in, dtype=np.float32)
    b_in = np.asarray(b_in, dtype=np.float32)
    W_out = np.asarray(W_out, dtype=np.float32)
    b_out = np.asarray(b_out, dtype=np.float32)

    use_bias = bool(np.any(b_in != 0))
    key = ("nc", use_bias)
    if key not in _CACHE:
        _CACHE[key] = _build_program(use_bias=use_bias)
    nc = _CACHE[key]

    in_maps = _host_prep(x, W_in, b_in, W_out)
    res = run_bass_kernel_spmd(nc, in_maps, core_ids=list(range(N_CORES)),
                               trace=_trace)
    LAST_RESULT = res

    out = np.empty((B, T, C), np.float32)
    for b in range(B):
        out[b] = (res.results[2 * b]["out"].astype(np.float32)
                  + res.results[2 * b + 1]["out"].astype(np.float32))
    if np.any(b_out != 0):
        out = out + b_out[None, None, :]
    return out



# revision 17
# speedup vs baseline: 1.0967x; 1.0734x over previous
"""Causal self-attention with interleaved RoPE on 8 Trainium2 NeuronCores, v2.

Problem: B=4, T=2048, C=1024, H=16, D=64 (fp32 reference).
  qkv = x @ W_in + b_in ; per-head interleaved RoPE on q,k ;
  causal softmax attention ; y @ W_out + b_out.

Sharding: core c <-> (batch b = c//2, head-half = c%2, 8 heads each).
COLLECTIVE-FREE: each core computes attention for its 8 heads over the
full T, normalizes locally (softmax denominators ride the PV matmul as a
ones-column of V), then computes a PARTIAL output projection using only
its heads' rows of W_out for ALL 2048 rows. The host adds the two
per-batch partials during unsharding (f32), which replaces the
AllGather/AllReduce entirely.

Structure (J-waves, J = query block of 512):
  wave J: reverse-project qT,kT (+RoPE via stream_shuffle partition
  swap) and forward-project V for t-rows [512J, 512J+512); attention for
  queries J over key blocks 0..4J+3 with combined 2-i-step [128,1024]
  exps; per-pair local normalize; partial out-projection of rows J.
  Projection/out-projection matmuls are spliced between attention
  i-pairs to keep the PE dense (HAM stays at K=8/8) while the scalar
  engine streams exps.

Layout notes:
 - qT/kT are produced TRANSPOSED directly (stationary = W chunk, moving
   = xT), so there are no PE transposes; RoPE pairs are host-permuted to
   [e0-15, o0-15, e16-31, o16-31] per head so the rotate-half partner
   sits +-16 partitions away inside one 32-partition quadrant
   (stream_shuffle mask [16..31,0..15]).
 - Scores are computed transposed (S^T [tk, tq]); exp without
   max-subtraction (|S|*scale <= ~6); diagonal blocks get full-width S,
   then memset+tri-mask on p (bf16).
 - Softmax denominators: ones-column in V -> PV psum row 64; normalize =
   cross-base row copy + bf16 reciprocal + sel-matmul broadcast + mul.
"""

import numpy as np

B, T, C, H = 4, 2048, 1024, 16
D = C // H            # 64
HPC = H // 2          # 8 heads per core
NP = HPC // 2         # 4 head-pairs per core
N_CORES = 8
ROPE_BASE = 10000.0
NW = 4                # 4 waves of 512 queries

_CACHE = {}

SHUF_MASK = list(range(16, 32)) + list(range(0, 16))


def _build_program(use_bias=False):
    import concourse.bass as bass
    import concourse.bacc as bacc
    import concourse.tile as tile
    import concourse.mybir as mybir

    f32 = mybir.dt.float32
    bf16 = mybir.dt.bfloat16
    Exp = mybir.ActivationFunctionType.Exp
    SCALE = 1.0 / float(np.sqrt(D))

    nc = bacc.Bacc("TRN2", target_bir_lowering=False, debug=False,
                   num_devices=N_CORES)

    xt_d = nc.dram_tensor("xt", [NW, 128, 8, 512], bf16, kind="ExternalInput")
    wqk_d = nc.dram_tensor("wqk", [128, 8, 1024], bf16, kind="ExternalInput")
    wv_d = nc.dram_tensor("wv", [128, 8, 512], bf16, kind="ExternalInput")
    wout_d = nc.dram_tensor("wout", [128, 4, 1024], bf16, kind="ExternalInput")
    cos_d = nc.dram_tensor("cosT", [128, T], bf16, kind="ExternalInput")
    sin_d = nc.dram_tensor("sinT", [128, T], bf16, kind="ExternalInput")
    tri_d = nc.dram_tensor("trimask", [128, 128], bf16, kind="ExternalInput")
    if use_bias:
        wqkb_d = nc.dram_tensor("wqkb", [128, 8], f32, kind="ExternalInput")
        wqkbs_d = nc.dram_tensor("wqkbs", [128, 8], f32, kind="ExternalInput")
        wvb_d = nc.dram_tensor("wvb", [1, 512], bf16, kind="ExternalInput")
    out_d = nc.dram_tensor("out", [T, C], bf16, kind="ExternalOutput")

    with tile.TileContext(nc) as tc:
        with (
            tc.tile_pool(name="g", bufs=1) as g,
            tc.tile_pool(name="xp", bufs=2) as xp,
            tc.tile_pool(name="rp", bufs=3) as rp,
            tc.tile_pool(name="pP", bufs=8) as pP,
            tc.tile_pool(name="yp", bufs=4) as yp,
            tc.tile_pool(name="op", bufs=2) as op,
            tc.tile_pool(name="psS", bufs=2, space="PSUM") as psS,
            tc.tile_pool(name="psV", bufs=2, space="PSUM") as psV,
            tc.tile_pool(name="psW", bufs=2, space="PSUM") as psW,
        ):
            # ---- constants (need-ordered: xt0 + first qk groups first) ----
            wqk_sb = g.tile([128, 8, 1024], bf16)
            xt_t = [None] * NW
            xt_t[0] = None  # placeholder; real alloc below
            cos_sb = g.tile([128, T], bf16)
            sin_sb = g.tile([128, T], bf16)
            wv_sb = g.tile([128, 8, 512], bf16)
            tri_sb = g.tile([128, 128], bf16)
            wout_sb = g.tile([128, 4, 1024], bf16)
            if use_bias:
                wqkb_sb = g.tile([128, 8], f32)
                nc.sync.dma_start(wqkb_sb[:], wqkb_d[:])
                wqkbs_sb = g.tile([128, 8], f32)
                nc.sync.dma_start(wqkbs_sb[:], wqkbs_d[:])
                wvb_sb = g.tile([1, 512], bf16)
                nc.sync.dma_start(wvb_sb[:], wvb_d[:])
                ones1 = g.tile([1, 128], bf16)
                nc.vector.memset(ones1[:], 1.0)

            # sel: rows 0/64 select broadcast sources for the two heads
            sel_sb = g.tile([128, 128], bf16)
            nc.vector.memset(sel_sb[:], 0.0)
            nc.vector.memset(sel_sb[0:1, 0:64], 1.0)
            nc.vector.memset(sel_sb[64:65, 64:128], 1.0)
            # r2: rows 0/64 hold denominators; other rows stay 1.0 so a
            # single reciprocal over [0:65] can't produce inf/nan
            r2_sb = g.tile([128, 512], f32)
            nc.vector.memset(r2_sb[:], 1.0)
            r2b_sb = g.tile([128, 512], bf16)
            nc.vector.memset(r2b_sb[:], 0.0)

            # ---- persistent activations ----
            qt_sb = g.tile([128, NP, T], bf16)
            kt_sb = g.tile([128, NP, T], bf16)
            v_sb = g.tile([128, 16, HPC, 65], bf16)
            nc.vector.memset(v_sb[:, :, :, 64:65], 1.0)

            def load_wave(J):
                xt_t[J] = xp.tile([128, 8, 512], bf16, tag="xt", name=f"xt{J}")
                nc.sync.dma_start(xt_t[J][:, 0:4, :], xt_d[J, :, 0:4, :])
                nc.sync.dma_start(xt_t[J][:, 4:8, :], xt_d[J, :, 4:8, :])
                ts = slice(J * 512, (J + 1) * 512)
                nc.sync.dma_start(cos_sb[:, ts], cos_d[:, ts])
                nc.sync.dma_start(sin_sb[:, ts], sin_d[:, ts])

            _qk_ps = {}

            def qk_group_a(J, gi):
                ps = psW.tile([128, 512], f32, tag="w", name=f"qk{J}_{gi}")
                _qk_ps[(J, gi)] = ps
                for kc in range(4):
                    nc.tensor.matmul(ps[:],
                                     wqk_sb[:, kc, gi * 128:(gi + 1) * 128],
                                     xt_t[J][:, kc, :],
                                     start=(kc == 0), stop=False)

            def qk_group_b(J, gi):
                ps = _qk_ps.pop((J, gi))
                for kc in range(4, 8):
                    nc.tensor.matmul(ps[:],
                                     wqk_sb[:, kc, gi * 128:(gi + 1) * 128],
                                     xt_t[J][:, kc, :],
                                     start=False, stop=(kc == 7))
                ts = slice(J * 512, (J + 1) * 512)
                m1 = rp.tile([128, 512], bf16, tag="m1", name="m1")
                tcos = rp.tile([128, 512], bf16, tag="tc", name="tcos")
                if use_bias:
                    sh = rp.tile([128, 512], f32, tag="sh", name="sh")
                    nc.vector.stream_shuffle(sh[:], ps[:], SHUF_MASK)
                    nc.vector.scalar_tensor_tensor(
                        tcos[:], ps[:], wqkb_sb[:, gi:gi + 1], cos_sb[:, ts],
                        op0=mybir.AluOpType.add, op1=mybir.AluOpType.mult)
                    nc.vector.scalar_tensor_tensor(
                        m1[:], sh[:], wqkbs_sb[:, gi:gi + 1], sin_sb[:, ts],
                        op0=mybir.AluOpType.add, op1=mybir.AluOpType.mult)
                else:
                    sh = rp.tile([128, 512], f32, tag="sh", name="sh")
                    nc.vector.stream_shuffle(sh[:], ps[:], SHUF_MASK)
                    nc.vector.tensor_mul(tcos[:], ps[:], cos_sb[:, ts])
                    nc.vector.tensor_mul(m1[:], sh[:], sin_sb[:, ts])
                dst = qt_sb[:, gi, ts] if gi < 4 else kt_sb[:, gi - 4, ts]
                nc.vector.tensor_add(dst, tcos[:], m1[:])

            def qk_group(J, gi):
                qk_group_a(J, gi)
                qk_group_b(J, gi)

            def v_chunk_a(J, ci):
                ps = psW.tile([128, 512], f32, tag="w", name=f"v{J}_{ci}")
                _qk_ps[("v", J, ci)] = ps
                for kc in range(4):
                    nc.tensor.matmul(ps[:],
                                     xt_t[J][:, kc, ci * 128:(ci + 1) * 128],
                                     wv_sb[:, kc, :],
                                     start=(kc == 0), stop=False)

            def v_chunk_b(J, ci):
                ps = _qk_ps.pop(("v", J, ci))
                for kc in range(4, 8):
                    nc.tensor.matmul(ps[:],
                                     xt_t[J][:, kc, ci * 128:(ci + 1) * 128],
                                     wv_sb[:, kc, :],
                                     start=False,
                                     stop=(kc == 7 and not use_bias))
                if use_bias:
                    nc.tensor.matmul(ps[:], ones1[0:1, :], wvb_sb[0:1, :],
                                     start=False, stop=True)
                tb = 4 * J + ci
                nc.vector.tensor_copy(
                    v_sb[:, tb, :, 0:64],
                    ps[:].rearrange("p (h d) -> p h d", d=64))

            def v_chunk(J, ci):
                v_chunk_a(J, ci)
                v_chunk_b(J, ci)

            y_t = [None] * NW

            def outproj_half(J, ci, lohi):
                """Partial out-proj for rows 512J+128ci, output half lohi."""
                ysl = y_t[J]
                pr = psW.tile([128, 512], f32, tag="w", name=f"pr{J}_{ci}_{lohi}")
                cs = slice(lohi * 512, (lohi + 1) * 512)
                for pp in range(NP):
                    nc.tensor.matmul(pr[:],
                                     ysl[:, pp, ci * 128:(ci + 1) * 128],
                                     wout_sb[:, pp, cs],
                                     start=(pp == 0), stop=(pp == NP - 1))
                ob = op.tile([128, 1024], bf16, tag="ob", name="ob")
                nc.vector.tensor_copy(ob[:, cs], pr[:])
                r0 = J * 512 + ci * 128
                nc.sync.dma_start(out_d[r0:r0 + 128, cs], ob[:, cs])
                return ob

            # out-proj halves must share the ob tile; emit as paired fillers
            _ob_pend = {}

            def outproj_chunk_fillers(J):
                fills = []
                for ci in range(4):
                    def f0(J=J, ci=ci):
                        _ob_pend[(J, ci)] = outproj_half(J, ci, 0)
                    def f1(J=J, ci=ci):
                        ysl = y_t[J]
                        ob = _ob_pend.pop((J, ci))
                        pr = psW.tile([128, 512], f32, tag="w",
                                      name=f"pr{J}_{ci}_1")
                        for pp in range(NP):
                            nc.tensor.matmul(pr[:],
                                             ysl[:, pp, ci * 128:(ci + 1) * 128],
                                             wout_sb[:, pp, 512:1024],
                                             start=(pp == 0), stop=(pp == NP - 1))
                        nc.vector.tensor_copy(ob[:, 512:1024], pr[:])
                        r0 = J * 512 + ci * 128
                        nc.sync.dma_start(out_d[r0:r0 + 128, 512:1024],
                                          ob[:, 512:1024])
                    fills.append(f0)
                    fills.append(f1)
                return fills

            pend_chain = []
            pend_norm = []

            def attn_wave(J, fillers, last=False):
                n_i = 4 * J + 4
                y_t[J] = yp.tile([128, NP, 512], bf16, tag="y", name=f"y{J}")
                fi = 0
                pend_pv = []
                tot_ipairs = NP * (n_i // 2)
                ip_idx = 1
                pop_at = min(2, n_i // 2 - 1)
                chain_at = max(0, pop_at - 1)
                for pp in range(NP):
                    pv = [psV.tile([128, 512], f32, tag="pv", name=f"pv{hh}")
                          for hh in range(2)]
                    for u in range(n_i // 2):
                        steps = (2 * u, 2 * u + 1)
                        di0 = 2 * u - 4 * J
                        sp = [psS.tile([128, 1024], f32, tag="s", name=f"s{hh}")
                              for hh in range(2)]
                        # hh outer so sp[0] completes 1 matmul earlier
                        for hh in range(2):
                            row = hh * 64
                            for ii, i in enumerate(steps):
                                di = i - 4 * J
                                c0 = di * 128 if di > 0 else 0
                                nc.tensor.matmul(
                                    sp[hh][:, ii * 512 + c0:(ii + 1) * 512],
                                    kt_sb[row:row + 64, pp, i * 128:(i + 1) * 128],
                                    qt_sb[row:row + 64, pp,
                                          J * 512 + c0:(J + 1) * 512],
                                    start=True, stop=True,
                                    tile_position=(row, 0))
                        p_t = [pP.tile([128, 1024], bf16, tag="p",
                                       name=f"p{hh}") for hh in range(2)]
                        for hh in range(2):
                            if di0 == 2:
                                # far-diagonal pair: only unmasked columns
                                nc.scalar.activation(p_t[hh][:, 256:512],
                                                     sp[hh][:, 256:512],
                                                     Exp, scale=SCALE)
                                nc.scalar.activation(p_t[hh][:, 896:1024],
                                                     sp[hh][:, 896:1024],
                                                     Exp, scale=SCALE)
                            else:
                                nc.scalar.activation(p_t[hh][:], sp[hh][:],
                                                     Exp, scale=SCALE)
                        for ii, i in enumerate(steps):
                            di = i - 4 * J
                            if di >= 0:
                                # tri-mask the 128-wide diagonal block
                                for hh in range(2):
                                    dsl = slice(ii * 512 + di * 128,
                                                ii * 512 + (di + 1) * 128)
                                    nc.gpsimd.tensor_mul(p_t[hh][:, dsl],
                                                         p_t[hh][:, dsl],
                                                         tri_sb[:])
                        # PV of the PREVIOUS i-pair: its exp+mask are
                        # long done, so the PE never stalls on them
                        if pend_pv:
                            pend_pv.pop(0)()
                        if u == chain_at and pend_chain:
                            pend_chain.pop(0)()
                        if u == pop_at and pend_norm:
                            pend_norm.pop(0)()
                        while (fi < len(fillers)
                               and fi * tot_ipairs <= ip_idx * len(fillers)):
                            fillers[fi]()
                            fi += 1
                        ip_idx += 1

                        def _pv(pp=pp, u=u, steps=steps, pv=pv, p_t=p_t,
                                n_i=n_i, J=J):
                            for ii, i in enumerate(steps):
                                di = i - 4 * J
                                c0 = di * 128 if di > 0 else 0
                                for hh in range(2):
                                    h = 2 * pp + hh
                                    nc.tensor.matmul(
                                        pv[hh][0:65, c0:512],
                                        v_sb[:, i, h, 0:65],
                                        p_t[hh][:, ii * 512 + c0:(ii + 1) * 512],
                                        start=(u == 0 and ii == 0),
                                        stop=(u == n_i // 2 - 1 and ii == 1))
                        pend_pv.append(_pv)
                    # ---- normalize pair pp (deferred emission: the
                    # sel-matmul lands after the next pair's first S block
                    # so the PE never stalls on the reciprocal chain) ----
                    def _norm_chain(J=J, pp=pp, pv=pv):
                        nc.vector.tensor_copy(r2_sb[0:1, :], pv[0][64:65, :])
                        nc.vector.tensor_copy(r2_sb[64:65, :], pv[1][64:65, :])
                        nc.vector.reciprocal_approx_fast(r2_sb[0:65, :],
                                                         r2_sb[0:65, :])
                        nc.vector.tensor_copy(r2b_sb[0:65, :], r2_sb[0:65, :])
                    def _norm_bcast(J=J, pp=pp, pv=pv):
                        rf = psW.tile([128, 512], f32, tag="w", name="rf")
                        nc.tensor.matmul(rf[:], sel_sb[0:65, :],
                                         r2b_sb[0:65, :],
                                         start=True, stop=True)
                        rfs = rp.tile([128, 512], bf16, tag="rfs", bufs=2,
                                      name="rfs")
                        nc.scalar.copy(rfs[:], rf[:])
                        nc.vector.tensor_mul(y_t[J][0:64, pp, :],
                                             pv[0][0:64, :], rfs[0:64, :])
                        nc.vector.tensor_mul(y_t[J][64:128, pp, :],
                                             pv[1][0:64, :], rfs[64:128, :])
                    pend_chain.append(_norm_chain)
                    pend_norm.append(_norm_bcast)
                # drain deferred PV, then leftover fillers; the last
                # pair's normalize carries into the NEXT wave (runway) --
                # except on the final wave, where it drains here
                while pend_pv:
                    pend_pv.pop(0)()
                if last:
                    while pend_chain:
                        pend_chain.pop(0)()
                while fi < len(fillers):
                    fillers[fi]()
                    fi += 1
                if last:
                    while pend_norm:
                        pend_norm.pop(0)()

            # ================= emission =================
            # wave 0: project only pair 0 + V upfront; pairs 1-3 become the
            # leading fillers of attn(0) (each pair's groups land one pair
            # ahead of use). outproj(J) runs as fillers in wave J+2 (y pool
            # bufs=3 allows the two-wave lag) so wave 3 stays fed.
            # head: need-ordered chunked DMAs so the first matmul can
            # start as soon as wqk g0/g4 chunks + xt0 kc chunks land.
            xt_t[0] = xp.tile([128, 8, 512], bf16, tag="xt", name="xt0")
            nc.sync.dma_start(wqk_sb[:, 0:4, 0:128], wqk_d[:, 0:4, 0:128])
            nc.sync.dma_start(xt_t[0][:, 0:4, :], xt_d[0, :, 0:4, :])
            nc.sync.dma_start(wqk_sb[:, 0:4, 512:640], wqk_d[:, 0:4, 512:640])
            nc.sync.dma_start(wqk_sb[:, 4:8, 0:128], wqk_d[:, 4:8, 0:128])
            nc.sync.dma_start(wqk_sb[:, 4:8, 512:640], wqk_d[:, 4:8, 512:640])
            nc.sync.dma_start(xt_t[0][:, 4:8, :], xt_d[0, :, 4:8, :])
            nc.sync.dma_start(cos_sb[:, 0:512], cos_d[:, 0:512])
            nc.sync.dma_start(sin_sb[:, 0:512], sin_d[:, 0:512])
            nc.sync.dma_start(wv_sb[:, 0:4, :], wv_d[:, 0:4, :])
            nc.sync.dma_start(wv_sb[:, 4:8, :], wv_d[:, 4:8, :])
            nc.sync.dma_start(tri_sb[:], tri_d[:])
            nc.sync.dma_start(wqk_sb[:, :, 128:512], wqk_d[:, :, 128:512])
            nc.sync.dma_start(wqk_sb[:, :, 640:1024], wqk_d[:, :, 640:1024])
            qk_group(0, 0)
            qk_group(0, 4)
            v_chunk(0, 0)
            v_chunk(0, 1)
            nc.sync.dma_start(wout_sb[:], wout_d[:])

            def proj_fillers(J):
                fills = []
                for gi in range(8):
                    fills.append(lambda J=J, gi=gi: qk_group_a(J, gi))
                    fills.append(lambda J=J, gi=gi: qk_group_b(J, gi))
                for ci in range(4):
                    fills.append(lambda J=J, ci=ci: v_chunk_a(J, ci))
                    fills.append(lambda J=J, ci=ci: v_chunk_b(J, ci))
                return fills

            for J in range(NW):
                fillers = []
                if J == 0:
                    for ci in (2, 3):
                        fillers.append(lambda ci=ci: v_chunk_a(0, ci))
                        fillers.append(lambda ci=ci: v_chunk_b(0, ci))
                    for gi in (1, 5, 2, 6, 3, 7):
                        fillers.append(lambda gi=gi: qk_group_a(0, gi))
                        fillers.append(lambda gi=gi: qk_group_b(0, gi))
                if J == NW - 1:
                    for Jo in range(NW - 1):
                        fillers += outproj_chunk_fillers(Jo)
                if J < NW - 1:
                    load_wave(J + 1)
                    fillers += proj_fillers(J + 1)
                attn_wave(J, fillers, last=(J == NW - 1))
            for f in outproj_chunk_fillers(NW - 1):
                f()

    nc.compile()
    return nc


def _host_prep(x, W_in, b_in, W_out):
    """Build per-core input maps."""
    import ml_dtypes

    bf = ml_dtypes.bfloat16

    # RoPE pair permutation: [e0..e15, o0..o15, e16..e31, o16..o31]
    perm = np.empty(D, np.int64)
    sign = np.empty(D, np.float64)
    fidx = np.empty(D, np.int64)
    for d in range(D):
        qd, w = d // 32, d % 32
        f = qd * 16 + (w % 16)
        perm[d] = 2 * f + (0 if w < 16 else 1)
        sign[d] = -1.0 if w < 16 else 1.0
        fidx[d] = f
    inv_freq = 1.0 / (ROPE_BASE ** (np.arange(0, D, 2, dtype=np.float64) / D))
    tpos = np.arange(T, dtype=np.float64)
    ang = tpos[None, :] * inv_freq[fidx][:, None]          # [64, T]
    cosT = np.tile(np.cos(ang), (2, 1)).astype(bf)  # [128, T]
    sinT = np.tile(sign[:, None] * np.sin(ang), (2, 1)).astype(bf)

    tri = (np.arange(128)[None, :] >= np.arange(128)[:, None]).astype(bf)

    in_maps = []
    for c in range(N_CORES):
        b, half = c // 2, c % 2
        heads = np.arange(half * HPC, (half + 1) * HPC)

        xt = np.ascontiguousarray(
            x[b].astype(bf).reshape(4, 512, 8, 128).transpose(0, 3, 2, 1))

        qk_cols = []
        for gi in range(4):
            hA, hB = heads[2 * gi], heads[2 * gi + 1]
            qk_cols.append(hA * D + perm)
            qk_cols.append(hB * D + perm)
        qcols = np.concatenate(qk_cols)
        kcols = C + qcols
        allqk = np.concatenate([qcols, kcols])
        wqk = np.ascontiguousarray(
            W_in[:, allqk].astype(bf).reshape(8, 128, 1024).transpose(1, 0, 2))
        vcols = np.concatenate([2 * C + h * D + np.arange(D) for h in heads])
        wv = np.ascontiguousarray(
            W_in[:, vcols].astype(bf).reshape(8, 128, 512).transpose(1, 0, 2))
        wout = np.ascontiguousarray(
            W_out[half * 512:(half + 1) * 512, :]
            .astype(bf).reshape(4, 128, 1024).transpose(1, 0, 2))

        m = {
            "xt": xt, "wqk": wqk, "wv": wv, "wout": wout,
            "cosT": cosT, "sinT": sinT, "trimask": tri,
        }
        if np.any(b_in != 0):
            qb = b_in[qcols]
            kb = b_in[kcols]
            wqkb = np.empty((128, 8), np.float32)
            wqkbs = np.empty((128, 8), np.float32)
            swap = np.concatenate([np.arange(16, 32), np.arange(0, 16),
                                   np.arange(48, 64), np.arange(32, 48)])
            for gi in range(8):
                bias = (qb if gi < 4 else kb)[(gi % 4) * 128:(gi % 4 + 1) * 128]
                wqkb[:, gi] = bias
                wqkbs[:, gi] = bias[swap]
            m["wqkb"] = wqkb
            m["wqkbs"] = wqkbs
            m["wvb"] = b_in[None, vcols].astype(bf)
        in_maps.append(m)
    return in_maps


LAST_RESULT = None


def kernel(x, W_in, b_in, W_out, b_out, _trace=False):
    global LAST_RESULT
    from concourse.bass_utils import run_bass_kernel_spmd

    x = np.asarray(x, dtype=np.float32)
    W_in = np.asarray(W_in, dtype=np.float32)
    b_in = np.asarray(b_in, dtype=np.float32)
    W_out = np.asarray(W_out, dtype=np.float32)
    b_out = np.asarray(b_out, dtype=np.float32)

    use_bias = bool(np.any(b_in != 0))
    key = ("nc", use_bias)
    if key not in _CACHE:
        _CACHE[key] = _build_program(use_bias=use_bias)
    nc = _CACHE[key]

    in_maps = _host_prep(x, W_in, b_in, W_out)
    res = run_bass_kernel_spmd(nc, in_maps, core_ids=list(range(N_CORES)),
                               trace=_trace)
    LAST_RESULT = res

    out = np.empty((B, T, C), np.float32)
    for b in range(B):
        out[b] = (res.results[2 * b]["out"].astype(np.float32)
                  + res.results[2 * b + 1]["out"].astype(np.float32))
    if np.any(b_out != 0):
        out = out + b_out[None, None, :]
    return out

